# revision 22
# baseline (speedup 1.0000x reference)
"""GQA attention (B=2, L=2048, D=2048, Hq=32, Hkv=8, hd=64) on 8 TRN2 cores.

Tensor-parallel over heads: core c owns q heads 4c..4c+3 and kv head c.
Each core computes a partial output (wo input-dim shard); host sums partials.

V2: all-bf16 matmul pipeline, merged QKV projection (3 full-width
M-blocks), DMA-XBAR transpose for V, causal trapezoid tiling (128-key
granularity inside 512-query blocks), fast reciprocal, PE software
pipelining (scores lead AV by 2; out-proj deferred one block).

Per-core layouts (feature-on-partition):
  xT      [2048, 4096]   x transposed bf16 (shared by all cores)
  wqkv_t  [2048, 384]    [wq shard (perm) | wk shard (perm) | wv shard] ^T
  wo_t    [256, 2048]    wo columns shard, transposed
  outT    [2048, 4096]   partial output bf16 (host: sum, T, reshape)
"""
import ml_dtypes
import numpy as np
from contextlib import ExitStack

import concourse.bass as bass
import concourse.mybir as mybir
import concourse.tile as tile
from concourse import bacc
from concourse.bass_utils import run_bass_kernel_spmd

F32 = mybir.dt.float32
F32R = mybir.dt.float32r
BF16 = mybir.dt.bfloat16
I32 = mybir.dt.int32
AF = mybir.ActivationFunctionType
ALU = mybir.AluOpType

B, L, D = 2, 2048, 2048
HQ, HKV, HD = 32, 8, 64
NCORES = 8
HL = HQ // NCORES          # 4 q heads per core
DQ = HL * HD               # 256 local q features
T = B * L                  # 4096 tokens
NB = 512                   # token block
NT = T // NB               # 8 token blocks
KC = D // 128              # 16 contraction chunks
ROPE_BASE = 10000.0
SCALE = 1.0 / np.sqrt(HD)

# stream_shuffle permutes per-partition within each 32-block (replicated
# every 32 rows): swap 16-row halves -> rope partner exchange
SHUF = [j ^ 16 for j in range(32)]
VCH = 96                   # vA chunk stride (64B-aligned dma transpose dest)

_CACHE = {}


def _build_module():
    nc = bacc.Bacc("TRN2", target_bir_lowering=False, debug=False,
                   num_devices=NCORES)

    d_xT = nc.dram_tensor("xT", [D, T], BF16, kind="ExternalInput").ap()
    d_wqkv = nc.dram_tensor("wqkv_t", [D, 384], BF16, kind="ExternalInput").ap()
    d_wo = nc.dram_tensor("wo_t", [DQ, D], BF16, kind="ExternalInput").ap()
    d_pos = nc.dram_tensor("pos", [1, L], I32, kind="ExternalInput").ap()
    d_invf = nc.dram_tensor("invf", [128, 1], F32, kind="ExternalInput").ap()
    d_eye = nc.dram_tensor("eye64", [64, 64], BF16, kind="ExternalInput").ap()
    d_out = nc.dram_tensor("outT", [D, T], BF16, kind="ExternalOutput").ap()

    with tile.TileContext(nc) as tc, ExitStack() as ctx, \
         nc.allow_low_precision(reason="bf16 matmul pipeline"):
        _kernel(tc, ctx, d_xT, d_wqkv, d_wo, d_pos, d_invf, d_eye, d_out)

    nc.compile()
    return nc


def _kernel(tc, ctx, d_xT, d_wqkv, d_wo, d_pos, d_invf, d_eye, d_out,
            dump=None):
    nc = tc.nc

    wpool = ctx.enter_context(tc.tile_pool(name="weights", bufs=1))
    spool = ctx.enter_context(tc.tile_pool(name="state", bufs=1))

    # ---------------- persistent SBUF tensors ----------------
    wqkvT = wpool.tile([128, KC * 384], BF16, tag="wqkvT")   # 12KB/part
    woT = wpool.tile([128, 2 * D], BF16, tag="woT")          # 8KB
    for kc in range(KC):
        nc.scalar.dma_start(wqkvT[:, kc * 384:(kc + 1) * 384],
                            d_wqkv[kc * 128:(kc + 1) * 128, :])
    for kc2 in range(2):
        nc.scalar.dma_start(woT[:, kc2 * D:(kc2 + 1) * D],
                            d_wo[kc2 * 128:(kc2 + 1) * 128, :])

    onesN = wpool.tile([33, 64], BF16, tag="onesN")
    nc.gpsimd.memset(onesN[:], 1.0)
    eye64 = wpool.tile([64, 64], BF16, tag="eye64")
    nc.scalar.dma_start(eye64[:], d_eye[:])

    # qT: [128, 2*T]; head pair p cols [p*T, (p+1)*T); even head rows 0:64,
    # odd head rows 64:128; within a head [even dims | odd dims].
    qT = spool.tile([128, 2 * T], BF16, tag="qT")            # 16KB
    # kT duplicated on rows 64:128 so odd-head matmuls get equal bases.
    kT = spool.tile([128, T], BF16, tag="kT")                # 8KB
    # v natural layout + ones column: chunk ch = cols [VCH*ch, VCH*ch+65)
    vA = spool.tile([128, 32 * VCH], BF16, tag="vA")         # 4.6KB
    nc.gpsimd.memset(vA[:], 1.0)  # ones columns; data cols overwritten
    # attention output, transposed: head pair tiles, b-major columns
    atP = [spool.tile([128, T], BF16, tag=f"atP{p}", name=f"atP{p}")
           for p in range(2)]                                # 16KB

    c128 = spool.tile([128, L], BF16, tag="c128")            # 4KB
    s128 = spool.tile([128, L], BF16, tag="s128")            # 4KB
    maskA = spool.tile([128, NB], BF16, tag="maskA")         # 1KB

    def build_trig_masks():
        # trig tables: rows 0:32/32:64/64:96/96:128 all hold the same [32]
        # invfreq set, so c128/s128 serve every 32-row band.
        with tc.tile_pool(name="trig", bufs=1) as trig:
            pos_i = trig.tile([1, L], I32, tag="sA", name="pos_i")
            nc.sync.dma_start(pos_i[:], d_pos[:])
            pos_f = trig.tile([1, L], F32, tag="sB", name="pos_f")
            nc.vector.tensor_copy(pos_f[:], pos_i[:])
            posb = trig.tile([128, L], F32, tag="sC", name="posb")
            nc.gpsimd.partition_broadcast(posb[:], pos_f[:])
            invf = trig.tile([128, 1], F32, tag="invf")
            nc.sync.dma_start(invf[:], d_invf[:])
            fq = trig.tile([128, L], F32, tag="sD", name="fq")
            nc.vector.tensor_scalar(fq[:], posb[:], invf[:], None, ALU.mult)
            # Cody-Waite range reduction, k via magic-number round-to-nearest
            INV2PI = float(np.float32(1.0 / (2 * np.pi)))
            C1 = float(np.float32(6.28125))
            C2 = float(np.float32(0.0019353071795864769))
            MAGIC = float(np.float32(12582912.0))            # 1.5 * 2**23
            t_a = trig.tile([128, L], F32, tag="sE", name="t_a")
            nc.vector.tensor_scalar(t_a[:], fq[:], INV2PI, None, ALU.mult)
            t_b = trig.tile([128, L], F32, tag="sC", name="t_b")
            nc.vector.tensor_scalar(t_b[:], t_a[:], MAGIC, None, ALU.add)
            kk = trig.tile([128, L], F32, tag="sB", name="kk")
            nc.vector.tensor_scalar(kk[:], t_b[:], MAGIC, None, ALU.subtract)
            nc.vector.tensor_scalar(t_a[:], kk[:], C1, None, ALU.mult)
            nc.vector.tensor_sub(fq[:], fq[:], t_a[:])
            nc.vector.tensor_scalar(t_a[:], kk[:], C2, None, ALU.mult)
            nc.vector.tensor_sub(fq[:], fq[:], t_a[:])       # reduced angle r
            nc.scalar.activation(s128[:], fq[:], AF.Sin)
            nc.scalar.activation(t_a[:], fq[:], AF.Abs)
            pi2 = trig.tile([128, 1], F32, tag="pi2")
            nc.gpsimd.memset(pi2[:], float(np.pi / 2))
            nc.scalar.activation(c128[:], t_a[:], AF.Sin, bias=pi2[:], scale=-1.0)
            # bake rotation signs into sin table: x1 (even-dim) rows get -sin
            for band in range(4):
                r0 = band * 32
                nc.vector.tensor_scalar(s128[r0:r0 + 16, :],
                                        s128[r0:r0 + 16, :],
                                        -1.0, None, ALU.mult)

        # causal mask tile (multiplicative): keep iff col j >= row i
        with tc.tile_pool(name="maskbuild", bufs=1) as mb:
            mones = mb.tile([128, NB], F32, tag="mones")
            nc.gpsimd.memset(mones[:], 1.0)
            masksf = mb.tile([128, NB], F32, tag="masksf")
            nc.gpsimd.affine_select(
                masksf[:], mones[:],
                pattern=[[1, NB]], compare_op=ALU.is_ge, fill=0.0,
                base=0, channel_multiplier=-1)
            nc.vector.tensor_copy(maskA[:], masksf[:])

    # ---------------- RoPE (DVE) ------------------------------------------
    tpool = ctx.enter_context(tc.tile_pool(name="tmp", bufs=2))

    def rope(dst, cols, l0):
        # y = x*cos + swap(x)*sgn*sin; swap = exchange 32-row halves per head
        xs_ = tpool.tile([128, NB], BF16, tag="ropeS")
        u = tpool.tile([128, NB], BF16, tag="ropeU")
        w = tpool.tile([128, NB], BF16, tag="ropeW")
        nc.vector.stream_shuffle(xs_[:], dst[:, cols], SHUF)
        nc.vector.tensor_mul(u[:], dst[:, cols], c128[:, l0:l0 + NB])
        nc.vector.tensor_mul(w[:], xs_[:], s128[:, l0:l0 + NB])
        nc.vector.tensor_add(dst[:, cols], u[:], w[:])

    def rope_nt(nt):
        b, l0 = nt // 4, (nt % 4) * NB
        for p in range(2):
            c0 = p * T + b * L + l0
            rope(qT, slice(c0, c0 + NB), l0)
        rope(kT, slice(b * L + l0, b * L + l0 + NB), l0)

    # ---------------- phase 1: merged QKV projection ----------------------
    with tc.tile_pool(name="xs", bufs=6) as xs, \
         tc.tile_pool(name="vst", bufs=2) as vst, \
         tc.tile_pool(name="pq0", bufs=2, space="PSUM") as pq0, \
         tc.tile_pool(name="pq1", bufs=2, space="PSUM") as pq1, \
         tc.tile_pool(name="pkv", bufs=2, space="PSUM") as pkv, \
         tc.tile_pool(name="ptp", bufs=2, space="PSUM") as ptp:
        trig_done = False
        pending_rope = []
        for nt in [0, 1, "trig", 2, 3, 4, 5, 6, 7]:
            if nt == "trig":
                build_trig_masks()
                trig_done = True
                for pnt in pending_rope:
                    rope_nt(pnt)
                pending_rope.clear()
                continue
            psq = [pq0.tile([128, NB], F32, tag="psq0", name=f"psq{nt}_0"),
                   pq1.tile([128, NB], F32, tag="psq1", name=f"psq{nt}_1")]
            pskv = pkv.tile([128, NB], F32, tag="pskv", name=f"pskv{nt}")
            for kc in range(KC):
                xk = xs.tile([128, NB], BF16, tag="xk")
                nc.sync.dma_start(
                    xk[:], d_xT[kc * 128:(kc + 1) * 128, nt * NB:(nt + 1) * NB])
                st, sp = kc == 0, kc == KC - 1
                for p in range(2):
                    nc.tensor.matmul(
                        psq[p][:],
                        wqkvT[:, kc * 384 + p * 128: kc * 384 + (p + 1) * 128],
                        xk[:], start=st, stop=sp)
                nc.tensor.matmul(pskv[:], wqkvT[:, kc * 384 + 256:(kc + 1) * 384],
                                 xk[:], start=st, stop=sp)
            # evictions (ACT copies cast fp32 -> bf16)
            for p in range(2):
                nc.scalar.copy(qT[:, p * T + nt * NB: p * T + (nt + 1) * NB],
                               psq[p][:])
            cols = slice(nt * NB, (nt + 1) * NB)
            nc.scalar.copy(kT[0:64, cols], pskv[0:64, :])
            # duplicate k rows via SBUF->SBUF DMA (no engine cost)
            nc.sync.dma_start(kT[64:128, cols], kT[0:64, cols])
            vstage = vst.tile([64, NB], BF16, tag="vstage", name=f"vst{nt}")
            nc.scalar.copy(vstage[:], pskv[64:128, :])
            for s in range(4):
                ch = nt * 4 + s
                tp_ps = ptp.tile([128, 64], BF16, tag="tp",
                                 name=f"tp{nt}_{s}")
                nc.tensor.transpose(tp_ps[:], vstage[:, s * 128:(s + 1) * 128],
                                    eye64[:])
                nc.vector.tensor_copy(vA[:, ch * VCH: ch * VCH + 64], tp_ps[:])
            if trig_done:
                rope_nt(nt)
            else:
                pending_rope.append(nt)

    # ---------------- phase 2+3: attention + out-projection ---------------
    with tc.tile_pool(name="epool", bufs=4) as ep, \
         tc.tile_pool(name="npool", bufs=2) as npool, \
         tc.tile_pool(name="opool", bufs=4) as op, \
         tc.tile_pool(name="pst", bufs=4, space="PSUM") as pst, \
         tc.tile_pool(name="pot", bufs=2, space="PSUM") as pot, \
         tc.tile_pool(name="pout", bufs=2, space="PSUM") as pout:

        def emit_outproj(nt):
            for mc in range(16):
                po = pout.tile([128, NB], F32, tag="po")
                nc.tensor.matmul(po[:], woT[:, mc * 128:(mc + 1) * 128],
                                 atP[0][:, nt * NB:(nt + 1) * NB],
                                 start=True, stop=False)
                nc.tensor.matmul(po[:], woT[:, D + mc * 128: D + (mc + 1) * 128],
                                 atP[1][:, nt * NB:(nt + 1) * NB],
                                 start=False, stop=True)
                osb = op.tile([128, NB], BF16, tag="osb")
                nc.vector.tensor_copy(osb[:], po[:])
                nc.sync.dma_start(
                    d_out[mc * 128:(mc + 1) * 128, nt * NB:(nt + 1) * NB],
                    osb[:])

        def emit_norm(state):
            # pair-batched normalization for a finished head pair
            p, b, qb, ot_e, ot_o = state
            cols = slice(b * L + qb * NB, b * L + (qb + 1) * NB)
            den2 = npool.tile([33, NB], F32, tag="den2")
            nc.vector.tensor_copy(den2[0:1, :], ot_e[64:65, :])
            nc.vector.tensor_copy(den2[32:33, :], ot_o[64:65, :])
            recip2 = npool.tile([33, NB], F32, tag="recip2")
            nc.vector.reciprocal_approx_fast(recip2[:], den2[:])
            recipb2 = npool.tile([33, NB], BF16, tag="recipb2")
            nc.vector.tensor_copy(recipb2[:], recip2[:])
            for par, ot_ps in ((0, ot_e), (1, ot_o)):
                bc_ps = pst.tile([128, NB], F32, tag="st",
                                 name=f"bc{b}_{qb}_{p}_{par}")
                nc.tensor.matmul(bc_ps[0:64, :], onesN[32 * par:32 * par + 1, :],
                                 recipb2[32 * par:32 * par + 1, :],
                                 start=True, stop=True)
                denb = npool.tile([64, NB], BF16, tag=f"denb{par}")
                nc.vector.tensor_copy(denb[:], bc_ps[0:64, :])
                nc.vector.tensor_mul(atP[p][64 * par:64 * par + 64, cols],
                                     ot_ps[0:64, :], denb[:])

        pending_norm = []
        pending_out = []
        for b in range(B):
            for qb in range(L // NB):
                ot_pair = []
                for h in range(HL):
                    p, odd = h // 2, h % 2
                    rbase = 64 * odd
                    qcols = p * T + b * L + qb * NB
                    ot_ps = pot.tile([65, NB], F32, tag="ot",
                                     name=f"ot{b}_{qb}_{h}")
                    nkb = 4 * (qb + 1)
                    # process diagonal blocks first so masks clear DVE early
                    order = list(range(4 * qb, nkb)) + list(range(4 * qb))
                    e_tiles = {}

                    def emit_scores(kb):
                        diag_c = kb - 4 * qb
                        col0 = 128 * diag_c if diag_c > 0 else 0
                        n = NB - col0
                        st_ps = pst.tile([128, NB], F32, tag="st",
                                         name=f"st{b}_{qb}_{h}_{kb}")
                        nc.tensor.matmul(
                            st_ps[:, col0:],
                            kT[rbase:rbase + 64,
                               b * L + kb * 128: b * L + (kb + 1) * 128],
                            qT[rbase:rbase + 64, qcols + col0: qcols + NB],
                            start=True, stop=True)
                        e_t = ep.tile([128, NB], BF16, tag="e")
                        nc.scalar.activation(e_t[:, col0:], st_ps[:, col0:],
                                             AF.Exp, scale=float(SCALE))
                        if diag_c >= 0:
                            nc.vector.tensor_mul(e_t[:, col0:], e_t[:, col0:],
                                                 maskA[:, 0:n])
                        e_tiles[kb] = (e_t, col0)

                    def emit_av(kb, first, last):
                        e_t, col0 = e_tiles.pop(kb)
                        ch = b * 16 + kb
                        nc.tensor.matmul(
                            ot_ps[:, col0:], vA[:, ch * VCH: ch * VCH + 65],
                            e_t[:, col0:], start=first, stop=last)

                    LAG = 3
                    for idx, kb in enumerate(order):
                        emit_scores(kb)
                        if idx == 1 and pending_norm:
                            emit_norm(pending_norm.pop())
                        if idx >= LAG:
                            emit_av(order[idx - LAG], idx - LAG == 0,
                                    idx - LAG == nkb - 1)
                    for idx in range(max(nkb - LAG, 0), nkb):
                        emit_av(order[idx], idx == 0, idx == nkb - 1)
                    ot_pair.append(ot_ps)
                    if odd:
                        pending_norm.append((p, b, qb, ot_pair[0], ot_pair[1]))
                        ot_pair = []
                # out-projection deferred one token block to hide norm latency
                pending_out.append(b * 4 + qb)
                if len(pending_out) > 1:
                    emit_outproj(pending_out.pop(0))
        while pending_norm:
            emit_norm(pending_norm.pop())
        while pending_out:
            emit_outproj(pending_out.pop(0))

    if dump is not None:
        for name, t in [("qT", qT), ("kT", kT), ("vA", vA),
                        ("atP0", atP[0]), ("atP1", atP[1]),
                        ("c128", c128), ("s128", s128),
                        ("maskA", maskA)]:
            if name not in dump:
                continue
            nc.sync.dma_start(dump[name][:], t[:])


ROPE_PERM = np.concatenate([np.arange(0, 32, 2), np.arange(1, 32, 2),
                            np.arange(32, 64, 2), np.arange(33, 64, 2)])


def _deinterleave_rows(w):
    # [H*64, D] -> per-head rows reordered so rope partners sit 16 apart
    # within each 32-row block: [e0..e15 | o0..o15 | e16..e31 | o16..o31]
    h = w.shape[0] // HD
    out = np.empty_like(w)
    for i in range(h):
        out[i * HD:(i + 1) * HD] = w[i * HD:(i + 1) * HD][ROPE_PERM]
    return out


def _prep_inputs(x, pos_ids, wq, wk, wv, wo):
    xT = np.ascontiguousarray(
        x.reshape(T, D).T).astype(ml_dtypes.bfloat16)
    pos = np.ascontiguousarray(pos_ids.astype(np.int32).reshape(1, L))
    half = HD // 2
    invf = (1.0 / (ROPE_BASE ** (np.arange(half, dtype=np.float32) / half)))
    # row r holds freq for lane (r%16) of half-block ((r%64)//32)
    idx = np.array([16 * ((r % 64) // 32) + (r % 16) for r in range(128)])
    invf128 = np.ascontiguousarray(invf[idx].reshape(128, 1))
    eye64 = np.eye(64).astype(ml_dtypes.bfloat16)
    in_maps = []
    for c in range(NCORES):
        wq_c = _deinterleave_rows(wq[c * DQ:(c + 1) * DQ])
        wk_c = _deinterleave_rows(wk[c * HD:(c + 1) * HD])
        wv_c = wv[c * HD:(c + 1) * HD]
        wqkv = np.concatenate([wq_c, wk_c, wv_c], axis=0)   # [384, D]
        wo_c = wo[:, c * DQ:(c + 1) * DQ]
        in_maps.append({
            "xT": xT,
            "wqkv_t": np.ascontiguousarray(wqkv.T).astype(ml_dtypes.bfloat16),
            "wo_t": np.ascontiguousarray(wo_c.T).astype(ml_dtypes.bfloat16),
            "pos": pos,
            "invf": invf128,
            "eye64": eye64,
        })
    return in_maps


def kernel(x, pos_ids, wq, wk, wv, wo, _trace=False):
    x = np.asarray(x)
    if "nc" not in _CACHE:
        _CACHE["nc"] = _build_module()
    nc = _CACHE["nc"]
    in_maps = _prep_inputs(np.asarray(x, np.float32), np.asarray(pos_ids),
                           np.asarray(wq, np.float32), np.asarray(wk, np.float32),
                           np.asarray(wv, np.float32), np.asarray(wo, np.float32))
    res = run_bass_kernel_spmd(nc, in_maps, core_ids=list(range(NCORES)),
                               trace=_trace)
    _CACHE["last_results"] = res
    acc = np.zeros((D, T), np.float32)
    for r in res.results:
        acc += r["outT"].astype(np.float32)
    return np.ascontiguousarray(acc.T).reshape(B, L, D)
